# revision 17
# baseline (speedup 1.0000x reference)
"""Trainium2 Bass kernel for a 4-layer LSTM-style stack with local+global logits.

Computation (per example row x of the [16384, 512] input):
    h0 = 0, c0 = 0
    for i in 1..4:
        z  = [x, h_{i-1}] @ W{f,i,o,c} + b        (4 gates, K = 1024)
        c  = tanh(z_c) * sigmoid(z_i) + sigmoid(z_f) * c
        h  = sigmoid(z_o) * tanh(c)
        local_i = h @ Wl_i + bl_i
    global = [x, h4] @ Wg + bg
Returns (concat(local_1..4) [16384, 960], global [16384, 960]).

Strategy (v2.1):
  - Data-parallel over 8 cores: 2048 rows each, weights replicated; each
    core processes its rows as 4 quarters of 512 examples.
  - Z = x @ W_top + b computed once per quarter (bf16), evacuated from PSUM
    by the Scalar engine (Identity + per-partition bias) into one contiguous
    [128, 8192] bf16 tile.  Layer 1 gates come straight from it.
  - Gate order is [i, c, f, o] (host reorders weight columns) so the merged
    activations are contiguous: sigmoid(i) | tanh(c) | sigmoid(f,o).
  - Layers 2..4 hidden-state matmuls run in fp8 (e4m3) DoubleRow perf mode:
    2 fp8 weights per PE cell, contraction 256 per matmul -> ~2x the bf16
    matmul rate.  h and W_bot are pre-scaled by 16; the 1/256 descale rides
    on the scalar_tensor_tensor that also adds Z:  pre = psum/256 + z.
  - h is kept in bf16 (feeds the bf16 logit matmuls, which must stay bf16
    for accuracy) and additionally cast to fp8*16 by the Scalar engine.
  - Local logits are computed TRANSPOSED (classes on partitions, Wl slices
    stationary, 32 wide matmuls per quarter instead of 256 narrow ones);
    the Scalar engine evacuates them with a per-class bias and the host
    transposes the [960, 2048] result back.
  - Global logits in natural layout with X/H stationary; DVE adds bias.
  - The emission plan interleaves the two in-flight quarters pair-by-pair
    inside each layer stage and places the PE-heavy Z/GL stages of later
    quarters right after DVE-heavy layer stages so the PE instruction
    stream never has to sit on a PSUM-recycle wait.
"""

import os
import sys

import numpy as np

for _p in ("/opt/trn_rl_repo", "/root/.axon_site/_ro/trn_rl_repo"):
    if os.path.isdir(_p) and _p not in sys.path:
        sys.path.insert(0, _p)

import ml_dtypes

import concourse.bass as bass
import concourse.tile as tile
from concourse import bacc, mybir
from concourse.bass_utils import run_bass_kernel_spmd

BF16 = mybir.dt.bfloat16
FP8 = mybir.dt.float8e4
F32 = mybir.dt.float32
AF = mybir.ActivationFunctionType
ALU = mybir.AluOpType
DR = mybir.MatmulPerfMode.DoubleRow

N_CORES = 8
N = 16384
K = 512                  # input features
U = 512                  # hidden units
GF = 4 * U               # 2048 concatenated gate features (order i, c, f, o)
MC = N // N_CORES        # 2048 rows per core
NQ = 4                   # quarters per core
EXQ = MC // NQ           # 512 examples per quarter
ET = EXQ // 128          # 4 example tiles of 128 per quarter
NCLS = [64, 128, 256, 512]
OFFS = [0, 64, 192, 448]
TOT = 960
N_LAYERS = 4

# local-logit slices in transposed layout: (goff, ncl, bias column)
LOC_SLICES = {
    0: [(0, 64, 0)],
    1: [(64, 128, 1)],
    2: [(192, 128, 2), (320, 128, 3)],
    3: [(448, 128, 4), (576, 128, 5), (704, 128, 6), (832, 128, 7)],
}

FP8_SCALE = 16.0         # pre-scale for h and W_bot before fp8 quantization
DESCALE = 1.0 / (FP8_SCALE * FP8_SCALE)

LAST_RESULT = None       # BassKernelResults of the most recent run (for test.py)


def _build_program():
    """Build the SPMD Bass program (identical on every core)."""
    nc = bacc.Bacc("TRN2", target_bir_lowering=False, debug=False)

    xt_d = nc.dram_tensor("xt", [K, MC], BF16, kind="ExternalInput")
    wtop_d = nc.dram_tensor("wtop", [K, GF], BF16, kind="ExternalInput")
    wdr_d = nc.dram_tensor("wdr", [128, 64, 128], FP8, kind="ExternalInput")
    wl_d = nc.dram_tensor("wl", [U, TOT], BF16, kind="ExternalInput")
    wg_d = nc.dram_tensor("wg", [K + U, TOT], BF16, kind="ExternalInput")
    bgate_d = nc.dram_tensor("bgate", [128, 16], F32, kind="ExternalInput")
    blt_d = nc.dram_tensor("blt", [128, 8], F32, kind="ExternalInput")
    bgrep_d = nc.dram_tensor("bgrep", [128, TOT], BF16, kind="ExternalInput")
    oloct_d = nc.dram_tensor("oloct", [TOT, MC], BF16, kind="ExternalOutput")
    oglb_d = nc.dram_tensor("oglb", [MC, TOT], BF16, kind="ExternalOutput")

    with tile.TileContext(nc) as tc:
        with (
            tc.tile_pool(name="wpool", bufs=1) as wpool,
            tc.tile_pool(name="xpool", bufs=4) as xpool,
            tc.tile_pool(name="zpool", bufs=4) as zpool,
            tc.tile_pool(name="pgp", bufs=2) as pgp,
            tc.tile_pool(name="hpool", bufs=5) as hpool,
            tc.tile_pool(name="h8pool", bufs=2) as h8pool,
            tc.tile_pool(name="cpool", bufs=2) as cpool,
            tc.tile_pool(name="tcp", bufs=1) as tcp,
            tc.tile_pool(name="ttp", bufs=1) as ttp,
            tc.tile_pool(name="lop", bufs=1) as lop,
            tc.tile_pool(name="glop", bufs=1) as glop,
            tc.tile_pool(name="zsum", bufs=2, space="PSUM") as zsum,
            tc.tile_pool(name="lgsum", bufs=2, space="PSUM") as lgsum,
            tc.tile_pool(name="drsum", bufs=2, space="PSUM") as drsum,
        ):
            # ---- resident weights/biases --------------------------------
            xs_pre = {}
            tiles = []
            wtop_sb = [[None] * 4 for _ in range(4)]   # [kt][gate block]
            for kt in range(4):
                t = xpool.tile([128, EXQ], BF16, tag=f"x{kt}")
                nc.sync.dma_start(t[:], xt_d[kt * 128:(kt + 1) * 128, 0:EXQ])
                tiles.append(t)
                w = wpool.tile([128, 512], BF16, tag=f"wt{kt}g0")
                nc.sync.dma_start(
                    w[:], wtop_d[kt * 128:(kt + 1) * 128, 0:512])
                wtop_sb[kt][0] = w
            xs_pre[0] = tiles
            bgate_sb = wpool.tile([128, 16], F32, tag="bgate")
            nc.sync.dma_start(bgate_sb[:], bgate_d[:])
            for g in range(1, 4):
                for kt in range(4):
                    t = wpool.tile([128, 512], BF16, tag=f"wt{kt}g{g}")
                    nc.sync.dma_start(
                        t[:], wtop_d[kt * 128:(kt + 1) * 128,
                                     g * 512:(g + 1) * 512])
                    wtop_sb[kt][g] = t
            tiles = []
            for kt in range(4):
                t = xpool.tile([128, EXQ], BF16, tag=f"x{kt}")
                nc.sync.dma_start(
                    t[:], xt_d[kt * 128:(kt + 1) * 128, EXQ:2 * EXQ])
                tiles.append(t)
            xs_pre[1] = tiles
            wdr_sb = wpool.tile([128, 64, 128], FP8, tag="wdr")
            nc.sync.dma_start(wdr_sb[:, :, :], wdr_d[:, :, :])
            wl_sb = []
            for kt in range(4):
                t = wpool.tile([128, TOT], BF16, tag=f"wl{kt}")
                nc.sync.dma_start(t[:], wl_d[kt * 128:(kt + 1) * 128, :])
                wl_sb.append(t)
            wg_sb = []
            for kt in range(8):
                t = wpool.tile([128, TOT], BF16, tag=f"wg{kt}")
                nc.sync.dma_start(t[:], wg_d[kt * 128:(kt + 1) * 128, :])
                wg_sb.append(t)
            blt_sb = wpool.tile([128, 8], F32, tag="blt")
            nc.sync.dma_start(blt_sb[:], blt_d[:])
            bgrep_sb = wpool.tile([128, TOT], BF16, tag="bgrep")
            nc.sync.dma_start(bgrep_sb[:], bgrep_d[:])

            # per-quarter live state
            xs = [None] * NQ      # 4 X^T tiles [128, EXQ] bf16
            zs = [None] * NQ      # Z' [128, 8192] bf16 (bias folded in)
            hs = [None] * NQ      # H [128, 2048] bf16 (current layer)
            h8s = [None] * NQ     # H fp8*16 [128, 4, 512] (current layer)
            cs = [None] * NQ      # C [128, 2048] bf16

            def z_closures(q):
                """Return one closure per Z of-group (usable as PE filler)."""
                if q in xs_pre:
                    xs[q] = xs_pre.pop(q)
                else:
                    xs[q] = []
                    for kt in range(4):
                        t = xpool.tile([128, EXQ], BF16, tag=f"x{kt}")
                        nc.sync.dma_start(
                            t[:], xt_d[kt * 128:(kt + 1) * 128,
                                       q * EXQ:(q + 1) * EXQ])
                        xs[q].append(t)
                zq = zpool.tile([128, 16 * EXQ], BF16, tag="z",
                                name=f"zq{q}")
                zs[q] = zq

                def emit_of(of):
                    g, t = of // 4, of % 4
                    ps = zsum.tile([128, EXQ], F32, tag="zs",
                                   name=f"z{q}of{of}")
                    for kt in range(4):
                        nc.tensor.matmul(
                            ps[:], wtop_sb[kt][g][:, t * 128:(t + 1) * 128],
                            xs[q][kt][:], start=(kt == 0), stop=(kt == 3))
                    nc.scalar.activation(
                        zq[:, of * EXQ:(of + 1) * EXQ], ps[:], AF.Identity,
                        bias=bgate_sb[:, of:of + 1], scale=1.0)
                return [(lambda of=of: emit_of(of)) for of in range(16)]

            def stage_z(q):
                """DMA this quarter's x, compute Z' = x @ W_top + b (bf16)."""
                for c in z_closures(q):
                    c()

            def cast_h8(q, hq):
                """hf8 = fp8(h * 16) for the next layer's DoubleRow matmul."""
                h8 = h8pool.tile([128, 4, EXQ], FP8, tag="h8")
                for kt in range(4):
                    nc.scalar.activation(
                        h8[:, kt, :], hq[:, kt * EXQ:(kt + 1) * EXQ],
                        AF.Copy, bias=0.0, scale=FP8_SCALE)
                h8s[q] = h8

            def emit_pair(q, j, pg):
                """DoubleRow matmuls + z-add for of-pair (2j, 2j+1)."""
                pp = drsum.tile([128, 2 * EXQ], F32, tag="dr")
                h8_prev = h8s[q]
                for p in range(2):      # contraction halves (kt 2p, 2p+1)
                    for o in range(2):
                        nc.tensor.matmul(
                            pp[:, o * EXQ:(o + 1) * EXQ],
                            wdr_sb[:, (2 * j + o) * 4 + p * 2:
                                   (2 * j + o) * 4 + p * 2 + 2, :],
                            h8_prev[:, 2 * p:2 * p + 2, :],
                            start=(p == 0), stop=(p == 1), perf_mode=DR)
                nc.vector.scalar_tensor_tensor(
                    pg[:, 2 * j * EXQ:(2 * j + 2) * EXQ], pp[:],
                    DESCALE, zs[q][:, 2 * j * EXQ:(2 * j + 2) * EXQ],
                    ALU.mult, ALU.add)

            def emit_acts(q, pg):
                """In-place activations pre -> gates on the pg tile."""
                nc.scalar.activation(
                    pg[:, 0:2048], pg[:, 0:2048], AF.Sigmoid)        # i
                nc.scalar.activation(
                    pg[:, 2048:4096], pg[:, 2048:4096], AF.Tanh)     # ch
                nc.scalar.activation(
                    pg[:, 4096:8192], pg[:, 4096:8192], AF.Sigmoid)  # f, o

            def cand_update(q, gacts, layer):
                """c = i*ch (+ f*c); h = o*tanh(c) on [128, 2048] tiles."""
                if layer == 1:
                    cn = cpool.tile([128, 2048], BF16, tag="c")
                    nc.vector.tensor_mul(
                        cn[:], gacts[:, 0:2048], gacts[:, 2048:4096])
                else:
                    t12 = ttp.tile([128, 4096], BF16, tag="t12")
                    nc.vector.tensor_mul(
                        t12[:, 0:2048], gacts[:, 0:2048], gacts[:, 2048:4096])
                    nc.vector.tensor_mul(
                        t12[:, 2048:4096], gacts[:, 4096:6144], cs[q][:])
                    cn = cpool.tile([128, 2048], BF16, tag="c")
                    nc.vector.tensor_add(
                        cn[:], t12[:, 0:2048], t12[:, 2048:4096])
                cs[q] = cn
                tc_t = tcp.tile([128, 2048], BF16, tag="tc")
                nc.scalar.activation(tc_t[:], cn[:], AF.Tanh)
                hn = hpool.tile([128, 2048], BF16, tag="h")
                nc.vector.tensor_mul(hn[:], gacts[:, 6144:8192], tc_t[:])
                hs[q] = hn
                if layer < 4:
                    cast_h8(q, hn)

            def stage_l1(q):
                """Layer 1: h0 = 0 so gates come straight from Z'."""
                ga = pgp.tile([128, 8192], BF16, tag="pg")
                zq = zs[q]
                nc.scalar.activation(
                    ga[:, 0:2048], zq[:, 0:2048], AF.Sigmoid)       # i
                nc.scalar.activation(
                    ga[:, 2048:4096], zq[:, 2048:4096], AF.Tanh)    # ch
                nc.scalar.activation(
                    ga[:, 6144:8192], zq[:, 6144:8192], AF.Sigmoid)  # o
                cand_update(q, ga, 1)

            def emit_local_slice(q, hq, goff, ncl, scol):
                """One transposed local-logit slice: Wl stationary."""
                ps = lgsum.tile([128, EXQ], F32, tag="lg")
                for kt in range(4):
                    nc.tensor.matmul(
                        ps[0:ncl, :], wl_sb[kt][:, goff:goff + ncl],
                        hq[:, kt * EXQ:(kt + 1) * EXQ],
                        start=(kt == 0), stop=(kt == 3))
                ot = lop.tile([128, EXQ], BF16, tag="lo")
                nc.vector.tensor_scalar(
                    ot[0:ncl, :], ps[0:ncl, :],
                    blt_sb[0:ncl, scol:scol + 1], None, ALU.add)
                nc.sync.dma_start(
                    oloct_d[goff:goff + ncl, q * EXQ:(q + 1) * EXQ],
                    ot[0:ncl, :])

            def loc_closures(qs, layer):
                """Transposed local-logit slices for layer (1-based) as
                filler closures; h must already be that layer's output."""
                out = []
                for q in qs:
                    hq = hs[q]
                    for (goff, ncl, scol) in LOC_SLICES[layer - 1]:
                        out.append(lambda q=q, hq=hq, g=goff, n=ncl, s=scol:
                                   emit_local_slice(q, hq, g, n, s))
                return out

            def stage_layer(qs, layer, fillers=()):
                """One layer for 1-2 quarters, pair-interleaved with filler
                closures (independent PE work) to cover PSUM-recycle waits."""
                fillers = list(fillers)
                pgs = {}
                for q in qs:
                    pgs[q] = pgp.tile([128, 8192], BF16, tag="pg",
                                      name=f"pg{q}l{layer}")
                fi = 0
                nfill = 2 if len(qs) == 1 else 1
                for j in range(8):
                    for q in qs:
                        emit_pair(q, j, pgs[q])
                    for _ in range(nfill):
                        if fi < len(fillers):
                            fillers[fi]()
                            fi += 1
                for q in qs:
                    emit_acts(q, pgs[q])
                for q in qs:
                    cand_update(q, pgs[q], layer)
                while fi < len(fillers):
                    fillers[fi]()
                    fi += 1

            def gl_xparts(q, es):
                """x-half of the global matmul: independent of h4, used as
                PE filler during the last layer's elementwise tail."""
                parts = []
                for e in es:
                    for s0, s1 in ((0, 512), (512, TOT)):
                        ps = lgsum.tile([128, EXQ], F32, tag="lg",
                                        name=f"glx{q}e{e}s{s0}")
                        w = s1 - s0
                        for kt in range(4):
                            nc.tensor.matmul(
                                ps[:, 0:w],
                                xs[q][kt][:, e * 128:(e + 1) * 128],
                                wg_sb[kt][:, s0:s1],
                                start=(kt == 0), stop=False)
                        parts.append((e, s0, s1, ps))
                return parts

            def gl_hparts(q, parts):
                hq = hs[q]
                gts = {}
                for (e, s0, s1, ps) in parts:
                    if e not in gts:
                        gts[e] = glop.tile([128, TOT], BF16, tag="glo",
                                           name=f"glh{q}e{e}")
                    w = s1 - s0
                    for kt in range(4, 8):
                        k2 = kt - 4
                        nc.tensor.matmul(
                            ps[:, 0:w],
                            hq[:, k2 * EXQ + e * 128:k2 * EXQ + (e + 1) * 128],
                            wg_sb[kt][:, s0:s1],
                            start=False, stop=(kt == 7))
                    nc.vector.tensor_add(
                        gts[e][:, s0:s1], ps[:, 0:w], bgrep_sb[:, s0:s1])
                for e, gt in gts.items():
                    r0 = q * EXQ + e * 128
                    nc.sync.dma_start(oglb_d[r0:r0 + 128, :], gt[:])

            def gl_unit(q, e):
                """global = [x, h4] @ Wg + bg for one example tile."""
                hq = hs[q]
                gt = glop.tile([128, TOT], BF16, tag="glo",
                               name=f"gl{q}e{e}")
                for s0, s1 in ((0, 512), (512, TOT)):
                    ps = lgsum.tile([128, EXQ], F32, tag="lg",
                                    name=f"gp{q}e{e}s{s0}")
                    w = s1 - s0
                    for kt in range(8):
                        if kt < 4:
                            st = xs[q][kt][:, e * 128:(e + 1) * 128]
                        else:
                            k2 = kt - 4
                            st = hq[:, k2 * EXQ + e * 128:
                                    k2 * EXQ + (e + 1) * 128]
                        nc.tensor.matmul(
                            ps[:, 0:w], st, wg_sb[kt][:, s0:s1],
                            start=(kt == 0), stop=(kt == 7))
                    nc.vector.tensor_add(
                        gt[:, s0:s1], ps[:, 0:w], bgrep_sb[:, s0:s1])
                r0 = q * EXQ + e * 128
                nc.sync.dma_start(oglb_d[r0:r0 + 128, :], gt[:])

            def gl_closures(q):
                return [(lambda e=e: gl_unit(q, e)) for e in range(ET)]

            def stage_gl(q):
                for c in gl_closures(q):
                    c()

            # ---- software-pipelined emission ----------------------------
            # Layer stages are DVE/ACT-heavy with idle PE windows; Z / local
            # / global matmul work of other quarters rides inside them as
            # filler closures interleaved with the DoubleRow pairs.
            # loc_closures capture h at creation time, so they are created
            # right when that layer's h is current and emitted later.
            stage_z(0)
            stage_l1(0)
            stage_z(1)
            stage_l1(1)
            loc01_l1 = loc_closures((0, 1), 1)
            stage_layer((0, 1), 2, z_closures(2) + loc01_l1)
            loc01_l2 = loc_closures((0, 1), 2)
            stage_layer((0, 1), 3, z_closures(3) + loc01_l2)
            loc01_l3 = loc_closures((0, 1), 3)
            stage_layer((0, 1), 4, loc01_l3)
            stage_l1(2)
            stage_l1(3)
            loc0_l4 = loc_closures((0,), 4)
            loc1_l4 = loc_closures((1,), 4)
            loc23_l1 = loc_closures((2, 3), 1)
            stage_layer((2, 3), 2, loc0_l4 + gl_closures(0) + loc23_l1)
            loc23_l2 = loc_closures((2, 3), 2)
            stage_layer((2, 3), 3, loc1_l4 + gl_closures(1) + loc23_l2)
            loc2_l3 = loc_closures((2,), 3)
            loc3_l3 = loc_closures((3,), 3)
            stage_layer((2,), 4, loc2_l3 + loc3_l3)
            loc2_l4 = loc_closures((2,), 4)
            stage_layer((3,), 4, loc2_l4 + gl_closures(2))
            for c in loc_closures((3,), 4) + gl_closures(3):
                c()

    nc.compile()
    return nc


_PROGRAM = None


def _get_program():
    global _PROGRAM
    if _PROGRAM is None:
        _PROGRAM = _build_program()
    return _PROGRAM


def _prep_weights(Wf, Wi, Wo, Wc, bf, bi, bo, bc,
                  Wl0, bl0, Wl1, bl1, Wl2, bl2, Wl3, bl3, Wg, bg):
    """Host-side packing: gate order [i, c, f, o]."""
    bf16 = ml_dtypes.bfloat16
    fp8 = ml_dtypes.float8_e4m3

    wcat = np.concatenate(
        [np.asarray(w, np.float32) for w in (Wi, Wc, Wf, Wo)],
        axis=1)                                       # [1024, 2048]
    bcat = np.concatenate(
        [np.asarray(b, np.float32) for b in (bi, bc, bf, bo)])  # [2048]
    wtop = np.ascontiguousarray(wcat[:K]).astype(bf16)          # [512, 2048]
    wbot = wcat[K:]                                             # [512, 2048]

    # DoubleRow weights: [128, 64, 128] fp8, dim1 = of*4 + p*2 + i with
    # value 16*Wbot[(2p+i)*128 + k, of*128 + m].
    wdr = np.empty((128, 64, 128), np.float32)
    for of in range(16):
        for p in range(2):
            for i in range(2):
                blk = wbot[(2 * p + i) * 128:(2 * p + i + 1) * 128,
                           of * 128:(of + 1) * 128]
                wdr[:, of * 4 + p * 2 + i, :] = blk
    wdr = np.clip(wdr * FP8_SCALE, -240.0, 240.0).astype(fp8)

    # bgate[128, 16]: column of holds bias for gate-feature block of
    bgate = np.ascontiguousarray(bcat.reshape(16, 128).T)

    wl = np.concatenate(
        [np.asarray(w, np.float32) for w in (Wl0, Wl1, Wl2, Wl3)],
        axis=1).astype(bf16)                          # [512, 960]
    blcat = np.concatenate([np.asarray(b, np.float32)
                            for b in (bl0, bl1, bl2, bl3)])     # [960]
    blt = np.zeros((128, 8), np.float32)
    col = 0
    for layer in range(4):
        for (goff, ncl, scol) in LOC_SLICES[layer]:
            blt[0:ncl, scol] = blcat[goff:goff + ncl]
            col += 1
    wg = np.asarray(Wg, np.float32).astype(bf16)      # [1024, 960]
    bgrep = np.ascontiguousarray(
        np.broadcast_to(np.asarray(bg, np.float32), (128, TOT))).astype(bf16)
    return dict(wtop=wtop, wdr=wdr, wl=wl, wg=wg,
                bgate=bgate, blt=blt, bgrep=bgrep)


def kernel(inputs, Wf, bf, Wi, bi, Wo, bo, Wc, bc,
           Wl0, bl0, Wl1, bl1, Wl2, bl2, Wl3, bl3, Wg, bg):
    global LAST_RESULT
    bf16 = ml_dtypes.bfloat16

    inputs = np.ascontiguousarray(np.asarray(inputs, dtype=np.float32))
    xt_all = inputs.T.astype(bf16)                    # [512, 16384]
    wmaps = _prep_weights(Wf, Wi, Wo, Wc, bf, bi, bo, bc,
                          Wl0, bl0, Wl1, bl1, Wl2, bl2, Wl3, bl3, Wg, bg)

    in_maps = []
    for c in range(N_CORES):
        m = {"xt": np.ascontiguousarray(xt_all[:, c * MC:(c + 1) * MC])}
        m.update(wmaps)
        in_maps.append(m)

    nc = _get_program()
    trace = os.environ.get("BASS_KERNEL_TRACE", "0") == "1"
    tmpdir = os.environ.get("BASS_KERNEL_TMPDIR") or None
    res = run_bass_kernel_spmd(
        nc, in_maps, list(range(N_CORES)), trace=trace, tmpdir=tmpdir)
    LAST_RESULT = res

    loc = np.concatenate(
        [np.asarray(r["oloct"], np.float32).T for r in res.results], axis=0)
    glb = np.concatenate(
        [np.asarray(r["oglb"], np.float32) for r in res.results], axis=0)
    return loc, glb


# revision 19
# speedup vs baseline: 1.1465x; 1.1465x over previous
"""Trainium2 Bass kernel for a 4-layer LSTM-style stack with local+global logits.

Computation (per example row x of the [16384, 512] input):
    h0 = 0, c0 = 0
    for i in 1..4:
        z  = [x, h_{i-1}] @ W{f,i,o,c} + b        (4 gates, K = 1024)
        c  = tanh(z_c) * sigmoid(z_i) + sigmoid(z_f) * c
        h  = sigmoid(z_o) * tanh(c)
        local_i = h @ Wl_i + bl_i
    global = [x, h4] @ Wg + bg
Returns (concat(local_1..4) [16384, 960], global [16384, 960]).

Strategy:
  - Data-parallel over 8 cores: 2048 rows each, weights replicated; each
    core processes its rows as 4 quarters of 512 examples.
  - Z = x @ W_top + b computed once per quarter (bf16), evacuated from PSUM
    by the Scalar engine (Identity + per-partition bias) into one contiguous
    [128, 8192] bf16 tile.  Layer 1 gates come straight from it.
  - Gate order is [i, c, f, o] (host reorders weight columns) so the merged
    activations are contiguous: sigmoid(i) | tanh(c) | sigmoid(f,o).
  - Layers 2..4 hidden-state matmuls run in fp8 (e4m3) DoubleRow perf mode:
    2 fp8 weights per PE cell, contraction 256 per matmul -> ~2x the bf16
    matmul rate.  h and W_bot are pre-scaled by 16 for fp8 range; the
    1/256 descale rides on the scalar_tensor_tensor that also adds Z:
        pre = (psum * 1/256) + z      (one DVE op per of-pair)
  - h is kept in bf16 (feeds the bf16 local/global logit matmuls, which
    must stay bf16 for accuracy) and additionally cast to fp8*16 by the
    Scalar engine for the next layer's DoubleRow matmul.
  - Candidate-cell elementwise math runs on [128, 2048] merged tiles
    (bf16, DVE 2x mode): c = i*ch + f*c, h = o*tanh(c).
  - Local/global logits in natural layout (examples on partitions) with the
    H/X tiles as stationary operands; bias added by DVE from replicated
    bias tiles; outputs stored bf16 and upcast to f32 on the host.
"""

import os
import sys

import numpy as np

for _p in ("/opt/trn_rl_repo", "/root/.axon_site/_ro/trn_rl_repo"):
    if os.path.isdir(_p) and _p not in sys.path:
        sys.path.insert(0, _p)

import ml_dtypes

import concourse.bass as bass
import concourse.tile as tile
from concourse import bacc, mybir
from concourse.bass_utils import run_bass_kernel_spmd

BF16 = mybir.dt.bfloat16
FP8 = mybir.dt.float8e4
F32 = mybir.dt.float32
AF = mybir.ActivationFunctionType
ALU = mybir.AluOpType
DR = mybir.MatmulPerfMode.DoubleRow

N_CORES = 8
N = 16384
K = 512                  # input features
U = 512                  # hidden units
GF = 4 * U               # 2048 concatenated gate features (order i, c, f, o)
MC = N // N_CORES        # 2048 rows per core
NQ = 4                   # quarters per core
EXQ = MC // NQ           # 512 examples per quarter
ET = EXQ // 128          # 4 example tiles of 128 per quarter
NCLS = [64, 128, 256, 512]
OFFS = [0, 64, 192, 448]
TOT = 960
N_LAYERS = 4

FP8_SCALE = 16.0         # pre-scale for h and W_bot before fp8 quantization
DESCALE = 1.0 / (FP8_SCALE * FP8_SCALE)

LAST_RESULT = None       # BassKernelResults of the most recent run (for test.py)


def _build_program():
    """Build the SPMD Bass program (identical on every core)."""
    nc = bacc.Bacc("TRN2", target_bir_lowering=False, debug=False)

    xt_d = nc.dram_tensor("xt", [K, MC], BF16, kind="ExternalInput")
    wtop_d = nc.dram_tensor("wtop", [K, GF], BF16, kind="ExternalInput")
    wdr_d = nc.dram_tensor("wdr", [128, 64, 128], FP8, kind="ExternalInput")
    wl_d = nc.dram_tensor("wl", [U, TOT], BF16, kind="ExternalInput")
    wg_d = nc.dram_tensor("wg", [K + U, TOT], BF16, kind="ExternalInput")
    bgate_d = nc.dram_tensor("bgate", [128, 16], F32, kind="ExternalInput")
    blrep_d = nc.dram_tensor("blrep", [128, TOT], F32, kind="ExternalInput")
    bgrep_d = nc.dram_tensor("bgrep", [128, TOT], F32, kind="ExternalInput")
    oloc_d = nc.dram_tensor("oloc", [MC, TOT], BF16, kind="ExternalOutput")
    oglb_d = nc.dram_tensor("oglb", [MC, TOT], BF16, kind="ExternalOutput")

    with tile.TileContext(nc) as tc:
        with (
            tc.tile_pool(name="wpool", bufs=1) as wpool,
            tc.tile_pool(name="xpool", bufs=3) as xpool,
            tc.tile_pool(name="zpool", bufs=2) as zpool,
            tc.tile_pool(name="pgp", bufs=2) as pgp,
            tc.tile_pool(name="hpool", bufs=3) as hpool,
            tc.tile_pool(name="h8pool", bufs=3) as h8pool,
            tc.tile_pool(name="cpool", bufs=2) as cpool,
            tc.tile_pool(name="tcp", bufs=2) as tcp,
            tc.tile_pool(name="ttp", bufs=2) as ttp,
            tc.tile_pool(name="lop", bufs=4) as lop,
            tc.tile_pool(name="glop", bufs=2) as glop,
            tc.tile_pool(name="zgsum", bufs=4, space="PSUM") as zgsum,
            tc.tile_pool(name="drsum", bufs=2, space="PSUM") as drsum,
        ):
            # ---- resident weights/biases --------------------------------
            xs_pre = {}
            tiles = []
            wtop_sb = [[None] * 4 for _ in range(4)]   # [kt][gate block]
            for kt in range(4):
                t = xpool.tile([128, EXQ], BF16, tag=f"x{kt}")
                nc.sync.dma_start(t[:], xt_d[kt * 128:(kt + 1) * 128, 0:EXQ])
                tiles.append(t)
                w = wpool.tile([128, 512], BF16, tag=f"wt{kt}g0")
                nc.sync.dma_start(
                    w[:], wtop_d[kt * 128:(kt + 1) * 128, 0:512])
                wtop_sb[kt][0] = w
            xs_pre[0] = tiles
            bgate_sb = wpool.tile([128, 16], F32, tag="bgate")
            nc.sync.dma_start(bgate_sb[:], bgate_d[:])
            for g in range(1, 4):
                for kt in range(4):
                    t = wpool.tile([128, 512], BF16, tag=f"wt{kt}g{g}")
                    nc.sync.dma_start(
                        t[:], wtop_d[kt * 128:(kt + 1) * 128,
                                     g * 512:(g + 1) * 512])
                    wtop_sb[kt][g] = t
            tiles = []
            for kt in range(4):
                t = xpool.tile([128, EXQ], BF16, tag=f"x{kt}")
                nc.sync.dma_start(
                    t[:], xt_d[kt * 128:(kt + 1) * 128, EXQ:2 * EXQ])
                tiles.append(t)
            xs_pre[1] = tiles
            wdr_sb = wpool.tile([128, 64, 128], FP8, tag="wdr")
            nc.sync.dma_start(wdr_sb[:, :, :], wdr_d[:, :, :])
            wl_sb = []
            for kt in range(4):
                t = wpool.tile([128, TOT], BF16, tag=f"wl{kt}")
                nc.sync.dma_start(t[:], wl_d[kt * 128:(kt + 1) * 128, :])
                wl_sb.append(t)
            wg_sb = []
            for kt in range(8):
                t = wpool.tile([128, TOT], BF16, tag=f"wg{kt}")
                nc.sync.dma_start(t[:], wg_d[kt * 128:(kt + 1) * 128, :])
                wg_sb.append(t)
            blrep_sb = wpool.tile([128, TOT], F32, tag="blrep")
            nc.sync.dma_start(blrep_sb[:], blrep_d[:])
            bgrep_sb = wpool.tile([128, TOT], F32, tag="bgrep")
            nc.sync.dma_start(bgrep_sb[:], bgrep_d[:])

            # per-quarter live state
            xs = [None] * NQ      # 4 X^T tiles [128, EXQ] bf16
            zs = [None] * NQ      # Z' [128, 8192] bf16 (bias folded in)
            hs = [None] * NQ      # H [128, 2048] bf16 (current layer)
            h8s = [None] * NQ     # H fp8*16 [128, 4, 512] (current layer)
            cs = [None] * NQ      # C [128, 2048] bf16

            def stage_z(q):
                """DMA this quarter's x, compute Z' = x @ W_top + b (bf16)."""
                if q in xs_pre:
                    xs[q] = xs_pre.pop(q)
                else:
                    xs[q] = []
                    for kt in range(4):
                        t = xpool.tile([128, EXQ], BF16, tag=f"x{kt}")
                        nc.sync.dma_start(
                            t[:], xt_d[kt * 128:(kt + 1) * 128,
                                       q * EXQ:(q + 1) * EXQ])
                        xs[q].append(t)
                zq = zpool.tile([128, 16 * EXQ], BF16, tag="z")
                for of in range(16):
                    g, t = of // 4, of % 4
                    ps = zgsum.tile([128, EXQ], F32, tag="zg")
                    for kt in range(4):
                        nc.tensor.matmul(
                            ps[:], wtop_sb[kt][g][:, t * 128:(t + 1) * 128],
                            xs[q][kt][:], start=(kt == 0), stop=(kt == 3))
                    nc.scalar.activation(
                        zq[:, of * EXQ:(of + 1) * EXQ], ps[:], AF.Identity,
                        bias=bgate_sb[:, of:of + 1], scale=1.0)
                zs[q] = zq

            def cast_h8(q, hq):
                """hf8 = fp8(h * 16) for the next layer's DoubleRow matmul."""
                h8 = h8pool.tile([128, 4, EXQ], FP8, tag="h8")
                for kt in range(4):
                    nc.scalar.activation(
                        h8[:, kt, :], hq[:, kt * EXQ:(kt + 1) * EXQ],
                        AF.Copy, bias=0.0, scale=FP8_SCALE)
                h8s[q] = h8

            def cand_update(q, gacts, layer):
                """c = i*ch (+ f*c); h = o*tanh(c) on [128, 2048] tiles."""
                if layer == 1:
                    cn = cpool.tile([128, 2048], BF16, tag="c")
                    nc.vector.tensor_mul(
                        cn[:], gacts[:, 0:2048], gacts[:, 2048:4096])
                else:
                    t12 = ttp.tile([128, 4096], BF16, tag="t12")
                    nc.vector.tensor_mul(
                        t12[:, 0:2048], gacts[:, 0:2048], gacts[:, 2048:4096])
                    nc.vector.tensor_mul(
                        t12[:, 2048:4096], gacts[:, 4096:6144], cs[q][:])
                    cn = cpool.tile([128, 2048], BF16, tag="c")
                    nc.vector.tensor_add(
                        cn[:], t12[:, 0:2048], t12[:, 2048:4096])
                cs[q] = cn
                tc_t = tcp.tile([128, 2048], BF16, tag="tc")
                nc.scalar.activation(tc_t[:], cn[:], AF.Tanh)
                hn = hpool.tile([128, 2048], BF16, tag="h")
                nc.vector.tensor_mul(hn[:], gacts[:, 6144:8192], tc_t[:])
                hs[q] = hn
                if layer < 4:
                    cast_h8(q, hn)

            def stage_l1(q):
                """Layer 1: h0 = 0 so gates come straight from Z'."""
                ga = pgp.tile([128, 8192], BF16, tag="pg")
                zq = zs[q]
                nc.scalar.activation(
                    ga[:, 0:2048], zq[:, 0:2048], AF.Sigmoid)       # i
                nc.scalar.activation(
                    ga[:, 2048:4096], zq[:, 2048:4096], AF.Tanh)    # ch
                nc.scalar.activation(
                    ga[:, 6144:8192], zq[:, 6144:8192], AF.Sigmoid)  # o
                cand_update(q, ga, 1)

            def emit_locals(q, layer, hq):
                """local_i = h_i @ Wl_i + bl_i, natural layout, DMA out."""
                off, ncl = OFFS[layer], NCLS[layer]
                for e in range(ET):
                    ps = zgsum.tile([128, EXQ], F32, tag="zg")
                    for kt in range(4):
                        nc.tensor.matmul(
                            ps[:, 0:ncl],
                            hq[:, kt * EXQ + e * 128:kt * EXQ + (e + 1) * 128],
                            wl_sb[kt][:, off:off + ncl],
                            start=(kt == 0), stop=(kt == 3))
                    ot = lop.tile([128, 512], BF16, tag="lo")
                    nc.vector.tensor_add(
                        ot[:, 0:ncl], ps[:, 0:ncl], blrep_sb[:, off:off + ncl])
                    r0 = q * EXQ + e * 128
                    nc.sync.dma_start(
                        oloc_d[r0:r0 + 128, off:off + ncl], ot[:, 0:ncl])

            def stage_layer(q, layer):
                """Layers 2..4: gates = act(Z + h @ W_bot); then locals of the
                previous layer (ready at the same time, keeps the PE busy)."""
                h_prev, h8_prev = hs[q], h8s[q]
                emit_locals(q, layer - 2, h_prev)
                pg = pgp.tile([128, 8192], BF16, tag="pg")
                zq = zs[q]
                for j in range(8):          # of-pair (2j, 2j+1)
                    pp = drsum.tile([128, 2 * EXQ], F32, tag="dr")
                    for p in range(2):      # contraction halves (kt 2p, 2p+1)
                        for o in range(2):
                            nc.tensor.matmul(
                                pp[:, o * EXQ:(o + 1) * EXQ],
                                wdr_sb[:, (2 * j + o) * 4 + p * 2:
                                       (2 * j + o) * 4 + p * 2 + 2, :],
                                h8_prev[:, 2 * p:2 * p + 2, :],
                                start=(p == 0), stop=(p == 1), perf_mode=DR)
                    nc.vector.scalar_tensor_tensor(
                        pg[:, 2 * j * EXQ:(2 * j + 2) * EXQ], pp[:],
                        DESCALE, zq[:, 2 * j * EXQ:(2 * j + 2) * EXQ],
                        ALU.mult, ALU.add)
                # in-place activations: pre -> gates
                nc.scalar.activation(
                    pg[:, 0:2048], pg[:, 0:2048], AF.Sigmoid)        # i
                nc.scalar.activation(
                    pg[:, 2048:4096], pg[:, 2048:4096], AF.Tanh)     # ch
                nc.scalar.activation(
                    pg[:, 4096:8192], pg[:, 4096:8192], AF.Sigmoid)  # f, o
                cand_update(q, pg, layer)

            def stage_gl(q):
                """locals of layer 4, then global = [x, h4] @ Wg + bg."""
                emit_locals(q, 3, hs[q])
                hq = hs[q]
                for e in range(ET):
                    gt = glop.tile([128, TOT], BF16, tag="glo")
                    for s0, s1 in ((0, 512), (512, TOT)):
                        ps = zgsum.tile([128, EXQ], F32, tag="zg")
                        w = s1 - s0
                        for kt in range(8):
                            if kt < 4:
                                st = xs[q][kt][:, e * 128:(e + 1) * 128]
                            else:
                                k2 = kt - 4
                                st = hq[:, k2 * EXQ + e * 128:
                                        k2 * EXQ + (e + 1) * 128]
                            nc.tensor.matmul(
                                ps[:, 0:w], st, wg_sb[kt][:, s0:s1],
                                start=(kt == 0), stop=(kt == 7))
                        nc.vector.tensor_add(
                            gt[:, s0:s1], ps[:, 0:w], bgrep_sb[:, s0:s1])
                    r0 = q * EXQ + e * 128
                    nc.sync.dma_start(oglb_d[r0:r0 + 128, :], gt[:])

            # ---- software-pipelined emission (2 quarters in flight) -----
            plan = [
                (0, "Z"), (1, "Z"), (0, "L1"), (1, "L1"),
                (0, 2), (1, 2), (0, 3), (1, 3), (0, 4), (1, 4),
                (0, "GL"), (2, "Z"), (2, "L1"), (1, "GL"),
                (3, "Z"), (3, "L1"),
                (2, 2), (3, 2), (2, 3), (3, 3), (2, 4), (3, 4),
                (2, "GL"), (3, "GL"),
            ]
            for q, s in plan:
                if s == "Z":
                    stage_z(q)
                elif s == "L1":
                    stage_l1(q)
                elif s == "GL":
                    stage_gl(q)
                else:
                    stage_layer(q, s)

    nc.compile()
    return nc


_PROGRAM = None


def _get_program():
    global _PROGRAM
    if _PROGRAM is None:
        _PROGRAM = _build_program()
    return _PROGRAM


def _prep_weights(Wf, Wi, Wo, Wc, bf, bi, bo, bc,
                  Wl0, bl0, Wl1, bl1, Wl2, bl2, Wl3, bl3, Wg, bg):
    """Host-side packing: gate order [i, c, f, o]."""
    bf16 = ml_dtypes.bfloat16
    fp8 = ml_dtypes.float8_e4m3

    wcat = np.concatenate(
        [np.asarray(w, np.float32) for w in (Wi, Wc, Wf, Wo)],
        axis=1)                                       # [1024, 2048]
    bcat = np.concatenate(
        [np.asarray(b, np.float32) for b in (bi, bc, bf, bo)])  # [2048]
    wtop = np.ascontiguousarray(wcat[:K]).astype(bf16)          # [512, 2048]
    wbot = wcat[K:]                                             # [512, 2048]

    # DoubleRow weights: [128, 64, 128] fp8, dim1 = of*4 + p*2 + i with
    # value 16*Wbot[(2p+i)*128 + k, of*128 + m].
    wdr = np.empty((128, 64, 128), np.float32)
    for of in range(16):
        for p in range(2):
            for i in range(2):
                blk = wbot[(2 * p + i) * 128:(2 * p + i + 1) * 128,
                           of * 128:(of + 1) * 128]
                wdr[:, of * 4 + p * 2 + i, :] = blk
    wdr = np.clip(wdr * FP8_SCALE, -240.0, 240.0).astype(fp8)

    # bgate[128, 16]: column of holds bias for gate-feature block of
    bgate = np.ascontiguousarray(bcat.reshape(16, 128).T)

    wl = np.concatenate(
        [np.asarray(w, np.float32) for w in (Wl0, Wl1, Wl2, Wl3)],
        axis=1).astype(bf16)                          # [512, 960]
    blrep = np.ascontiguousarray(np.broadcast_to(
        np.concatenate([np.asarray(b, np.float32)
                        for b in (bl0, bl1, bl2, bl3)]), (128, TOT)))
    wg = np.asarray(Wg, np.float32).astype(bf16)      # [1024, 960]
    bgrep = np.ascontiguousarray(
        np.broadcast_to(np.asarray(bg, np.float32), (128, TOT)))
    return dict(wtop=wtop, wdr=wdr, wl=wl, wg=wg,
                bgate=bgate, blrep=blrep, bgrep=bgrep)


def kernel(inputs, Wf, bf, Wi, bi, Wo, bo, Wc, bc,
           Wl0, bl0, Wl1, bl1, Wl2, bl2, Wl3, bl3, Wg, bg):
    global LAST_RESULT
    bf16 = ml_dtypes.bfloat16

    inputs = np.ascontiguousarray(np.asarray(inputs, dtype=np.float32))
    xt_all = inputs.T.astype(bf16)                    # [512, 16384]
    wmaps = _prep_weights(Wf, Wi, Wo, Wc, bf, bi, bo, bc,
                          Wl0, bl0, Wl1, bl1, Wl2, bl2, Wl3, bl3, Wg, bg)

    in_maps = []
    for c in range(N_CORES):
        m = {"xt": np.ascontiguousarray(xt_all[:, c * MC:(c + 1) * MC])}
        m.update(wmaps)
        in_maps.append(m)

    nc = _get_program()
    trace = os.environ.get("BASS_KERNEL_TRACE", "0") == "1"
    tmpdir = os.environ.get("BASS_KERNEL_TMPDIR") or None
    res = run_bass_kernel_spmd(
        nc, in_maps, list(range(N_CORES)), trace=trace, tmpdir=tmpdir)
    LAST_RESULT = res

    loc = np.concatenate(
        [np.asarray(r["oloc"], np.float32) for r in res.results], axis=0)
    glb = np.concatenate(
        [np.asarray(r["oglb"], np.float32) for r in res.results], axis=0)
    return loc, glb
